# revision 7
# baseline (speedup 1.0000x reference)
"""DynamicPooler (nn_DynamicPooler_58540404244695) Trainium2 Bass kernel.

Full-input contract: kernel(queries, keys, values) takes the complete
(L=2048, B=4, Q_H=32/KV_H=8, D=128) fp32 tensors and returns the full
(q_out, k2, v2, mask) tuple, matching the reference module.

Sharding: B*KV_H = 32 independent sequences split across 8 NeuronCores
(4 per core: one batch element x 4 contiguous kv heads). The poolsum
recurrence runs along L and is independent across sequences, so there is
no cross-core communication.

Per-core algorithm (t on partitions, 16 tiles of 128 tokens):
  gates   g[t,n] = (keys[t,n,127] > 2)    — straight-through router, exact 0/1
  weights w[t,n] = sigmoid(mean_g(q_last) + 2) + eps
  x = [k | v | 1]  (L, 4, 257)            — ones column yields the denominator
  out[t] = sum_{m=0..11} P[m,t] w[t-m] x[t-m] / sum_m P[m,t] w[t-m]
  where P[m,t] = prod_{d=1..m} g[t-d+1] is binary, so with
  Zc = cumsum(1-g):  P[m,t] = (Zc[t] == Zc[t-m]).
  The windowed sum is a banded matmul on the TensorEngine:
      out_tile = lhsT_diag.T @ x_tile + lhsT_prev.T @ x_prev
  with lhsT[q,p] = (Zc[p] == Zc[q], within band) * w[q]. Band masks are
  folded into the broadcast Zc matrix as +4096 out-of-band penalties
  (Zc <= 2048 < 4096, so equality can never hold out of band).
  GQA expansion (x4 heads) happens in the output DMA addressing; the mask's
  t+1 shift happens in the mask DMA addressing.
"""

import numpy as np

from concourse import bacc, bass, tile
import concourse.mybir as mybir
from concourse.bass_utils import run_bass_kernel_spmd

F32 = mybir.dt.float32
ALU = mybir.AluOpType
AFT = mybir.ActivationFunctionType

EPS = 1.1920929e-07
MASK_CONST = -50000.0
WS = 12
PEN = 4096.0

L, B, Q_H, KV_H, D = 2048, 4, 32, 8, 128
N_CORES = 8


def _emit_core_kernel(tc, io, L=2048):
    nc = tc.nc
    T = L // 128
    q_dram, kv_dram = io["q"], io["kv"]
    qo_dram, k2_dram, v2_dram, mask_dram = (
        io["q_out"], io["k2"], io["v2"], io["mask"])

    with tc.tile_pool(name="const", bufs=1) as cpool, \
         tc.tile_pool(name="persist", bufs=1) as pers, \
         tc.tile_pool(name="stream", bufs=3) as stream, \
         tc.tile_pool(name="small", bufs=4) as small, \
         tc.tile_pool(name="ps", bufs=1, space="PSUM") as pp:

        # ---- constants ----
        # walrus codegen only implements is_ge/is_gt for affine_select, so
        # every mask below is built from is_ge pairs.
        ident128 = cpool.tile([128, 128], F32)
        nc.vector.memset(ident128[:], 1.0)
        nc.gpsimd.affine_select(ident128[:], ident128[:], pattern=[[-1, 128]],
                                compare_op=ALU.is_ge, fill=0.0,
                                base=0, channel_multiplier=1)
        nc.gpsimd.affine_select(ident128[:], ident128[:], pattern=[[1, 128]],
                                compare_op=ALU.is_ge, fill=0.0,
                                base=0, channel_multiplier=-1)
        ident1 = cpool.tile([1, 1], F32)
        nc.vector.memset(ident1[:], 1.0)
        ones_row = cpool.tile([1, 128], F32)
        nc.vector.memset(ones_row[:], 1.0)
        zeros_row = cpool.tile([1, 16], F32)
        nc.vector.memset(zeros_row[:], 0.0)
        bias_m2 = cpool.tile([128, 1], F32)
        nc.vector.memset(bias_m2[:], -2.0)
        # 4-block identity: I4[k, n*128+p] = (k == p)
        ident4blk = cpool.tile([128, 512], F32)
        nc.vector.memset(ident4blk[:], 1.0)
        nc.gpsimd.affine_select(
            ident4blk[:].rearrange("k (n p) -> k n p", n=4),
            ident4blk[:].rearrange("k (n p) -> k n p", n=4),
            pattern=[[0, 4], [-1, 128]], compare_op=ALU.is_ge, fill=0.0,
            base=0, channel_multiplier=1)
        nc.gpsimd.affine_select(
            ident4blk[:].rearrange("k (n p) -> k n p", n=4),
            ident4blk[:].rearrange("k (n p) -> k n p", n=4),
            pattern=[[0, 4], [1, 128]], compare_op=ALU.is_ge, fill=0.0,
            base=0, channel_multiplier=-1)
        # diag band penalty (transposed): penT_d[p, q] = 0 iff 0 <= p-q <= 11
        penT_d = cpool.tile([128, 128], F32)
        nc.vector.memset(penT_d[:], 0.0)
        nc.gpsimd.affine_select(penT_d[:], penT_d[:], pattern=[[-1, 128]],
                                compare_op=ALU.is_ge, fill=PEN,
                                base=0, channel_multiplier=1)
        nc.gpsimd.affine_select(penT_d[:], penT_d[:], pattern=[[1, 128]],
                                compare_op=ALU.is_ge, fill=PEN,
                                base=WS - 1, channel_multiplier=-1)
        # prev band penalty (transposed): penT_p[p, q] = 0 iff q-p >= 117
        penT_p = cpool.tile([128, 128], F32)
        nc.vector.memset(penT_p[:], 0.0)
        nc.gpsimd.affine_select(penT_p[:], penT_p[:], pattern=[[1, 128]],
                                compare_op=ALU.is_ge, fill=PEN,
                                base=-(128 - (WS - 1)), channel_multiplier=-1)

        # ---- persistent tiles ----
        omgT = pers.tile([4, L], F32)        # (1-g) transposed
        Zc4 = pers.tile([4, L], F32)         # cumsum(1-g)
        Zc_cat = pers.tile([1, 4 * L], F32)  # n-major 128-blocks
        mask_sb = pers.tile([128, T * 16], F32)

        with tc.tile_pool(name="xp", bufs=T) as xpool, \
             tc.tile_pool(name="wp", bufs=T) as wpool, \
             tc.tile_pool(name="zp", bufs=8) as zpool:

            x_tiles, w_tiles = [], []
            # ================= pass 1 =================
            for i in range(T):
                q_t = stream.tile([128, 2048], F32, tag="qt")
                nc.sync.dma_start(
                    out=q_t[:].rearrange("p (h d) -> p h d", d=128),
                    in_=q_dram[128 * i:128 * (i + 1)])
                x_t = xpool.tile([128, 1028], F32, tag="xt")
                nc.sync.dma_start(
                    out=x_t[:].rearrange("p (n c) -> p n c", c=257),
                    in_=kv_dram[128 * i:128 * (i + 1)])
                x_tiles.append(x_t)

                # omg = 1 - gate = (k_last <= 2)
                omg_t = small.tile([128, 4], F32, tag="omg")
                kcol = x_t[:].rearrange("p (n c) -> p n c", c=257)[:, :, 127]
                nc.vector.tensor_scalar(omg_t[:], kcol, 2.0, None,
                                        op0=ALU.is_le)

                # mask block (unshifted): g * -50000 = omg*50000 - 50000
                mblk = mask_sb[:, 16 * i:16 * (i + 1)].rearrange(
                    "p (n j) -> p n j", j=4)
                omg_b = omg_t[:].unsqueeze(2).broadcast_to([128, 4, 4])
                nc.vector.tensor_scalar(mblk, omg_b, 50000.0, MASK_CONST,
                                        op0=ALU.mult, op1=ALU.add)

                # transpose omg -> omgT columns
                gT_ps = pp.tile([4, 128], F32, tag="tp", bufs=2)
                nc.tensor.transpose(gT_ps[:], omg_t[:], ident128[:])
                nc.scalar.copy(omgT[:, 128 * i:128 * (i + 1)], gT_ps[:])

                # w = 1/(1+exp(-(0.25*s+2))) + EPS
                s_t = small.tile([128, 4], F32, tag="s")
                qlast = q_t[:].rearrange(
                    "p (n g d) -> p n g d", g=4, d=128)[:, :, :, 127]
                nc.vector.tensor_reduce(s_t[:], qlast,
                                        axis=mybir.AxisListType.X, op=ALU.add)
                e_t = small.tile([128, 4], F32, tag="e")
                nc.scalar.activation(e_t[:], s_t[:], AFT.Exp,
                                     bias=bias_m2[:], scale=-0.25)
                d_t = small.tile([128, 4], F32, tag="d")
                nc.vector.tensor_scalar(d_t[:], e_t[:], 1.0, None, op0=ALU.add)
                r_t = small.tile([128, 4], F32, tag="r")
                nc.vector.reciprocal(r_t[:], d_t[:])
                w_t = wpool.tile([128, 4], F32, tag="wt")
                nc.vector.tensor_scalar(w_t[:], r_t[:], EPS, None, op0=ALU.add)
                w_tiles.append(w_t)

                # zero last channel of all q heads, store
                nc.vector.memset(
                    q_t[:].rearrange("p (h d) -> p h d", d=128)[:, :, 127:128],
                    0.0)
                nc.scalar.dma_start(
                    out=qo_dram[128 * i:128 * (i + 1)],
                    in_=q_t[:].rearrange("p (h d) -> p h d", d=128))

            # ================= scan =================
            nc.vector.tensor_tensor_scan(Zc4[:], omgT[:], omgT[:], 0.0,
                                         op0=ALU.add, op1=ALU.bypass)
            nc.sync.dma_start(
                out=Zc_cat[:].rearrange("o (n i p) -> o n i p", n=4, p=128),
                in_=Zc4[:].rearrange("n (i p) -> n i p", p=128))

            # ================= pass 2 =================
            prev_zcts = None
            for i in range(T):
                zc_slice = Zc_cat[:].rearrange(
                    "o (n i p) -> o n i p", n=4, p=128)[:, :, i, :]

                zcb_d = pp.tile([128, 512], F32, tag="zcb", bufs=2)
                nc.tensor.matmul(zcb_d[:], ones_row[:], zc_slice,
                                 start=True, stop=False)
                nc.tensor.matmul(zcb_d[:], penT_d[:], ident4blk[:],
                                 start=False, stop=True)
                if i > 0:
                    zcb_p = pp.tile([128, 512], F32, tag="zcb", bufs=2)
                    nc.tensor.matmul(zcb_p[:], ones_row[:], zc_slice,
                                     start=True, stop=False)
                    nc.tensor.matmul(zcb_p[:], penT_p[:], ident4blk[:],
                                     start=False, stop=True)

                okv_t = stream.tile([128, 1024], F32, tag="okv")
                cur_zcts = []
                for n in range(4):
                    zcT_ps = pp.tile([128, 1], F32, tag="tp", bufs=2)
                    nc.tensor.transpose(
                        zcT_ps[:],
                        Zc_cat[0:1, 128 * (n * T + i):128 * (n * T + i) + 128],
                        ident1[:])
                    zcT_t = zpool.tile([128, 1], F32, tag="zct")
                    nc.scalar.copy(zcT_t[:], zcT_ps[:])
                    cur_zcts.append(zcT_t)

                    ps_n = pp.tile([128, 257], F32, tag="mm", bufs=4)
                    if i > 0:
                        lhsT_p = small.tile([128, 128], F32, tag="lhsT",
                                            bufs=4)
                        nc.vector.tensor_scalar(
                            lhsT_p[:], zcb_p[:, 128 * n:128 * (n + 1)],
                            prev_zcts[n][:], w_tiles[i - 1][:, n:n + 1],
                            op0=ALU.is_equal, op1=ALU.mult)
                        nc.tensor.matmul(
                            ps_n[:], lhsT_p[:],
                            x_tiles[i - 1][:, 257 * n:257 * (n + 1)],
                            start=True, stop=False)
                    lhsT_d = small.tile([128, 128], F32, tag="lhsT", bufs=4)
                    nc.vector.tensor_scalar(
                        lhsT_d[:], zcb_d[:, 128 * n:128 * (n + 1)],
                        zcT_t[:], w_tiles[i][:, n:n + 1],
                        op0=ALU.is_equal, op1=ALU.mult)
                    nc.tensor.matmul(ps_n[:], lhsT_d[:],
                                     x_tiles[i][:, 257 * n:257 * (n + 1)],
                                     start=(i == 0), stop=True)

                    rc_n = small.tile([128, 1], F32, tag="rc")
                    nc.vector.reciprocal(rc_n[:], ps_n[:, 256:257])
                    nc.scalar.activation(okv_t[:, 256 * n:256 * (n + 1)],
                                         ps_n[:, 0:256], AFT.Copy,
                                         scale=rc_n[:])
                prev_zcts = cur_zcts
                # zero last channel of pooled k
                nc.vector.memset(
                    okv_t[:].rearrange("p (n c) -> p n c", c=256)[:, :, 127:128],
                    0.0)
                # GQA-expanded stores: one DMA per replicated head j
                okv_v = okv_t[:].rearrange("p (n c) -> p n c", c=256)
                k2_v = k2_dram[128 * i:128 * (i + 1)].rearrange(
                    "t (n j) d -> t n j d", j=4)
                v2_v = v2_dram[128 * i:128 * (i + 1)].rearrange(
                    "t (n j) d -> t n j d", j=4)
                for j in range(4):
                    nc.scalar.dma_start(out=k2_v[:, :, j, :],
                                        in_=okv_v[:, :, 0:128])
                    nc.scalar.dma_start(out=v2_v[:, :, j, :],
                                        in_=okv_v[:, :, 128:256])

            # ============ mask stores (t+1 shift via addressing) ============
            mview = mask_dram[:].rearrange("(i p) c -> p i c", p=128)
            nc.scalar.dma_start(
                out=mview[0:127, :, :],
                in_=mask_sb[1:128, :].rearrange("p (i c) -> p i c", c=16))
            if T > 1:
                nc.scalar.dma_start(
                    out=mview[127:128, 0:T - 1, :],
                    in_=mask_sb[0:1, 16:].rearrange("p (i c) -> p i c", c=16))
            nc.scalar.dma_start(
                out=mview[127:128, T - 1:T, :],
                in_=zeros_row[:].unsqueeze(1))


_PROGRAM = None


def _build_program():
    global _PROGRAM
    if _PROGRAM is not None:
        return _PROGRAM
    # Bacc (not raw Bass): its compile pipeline runs generate_event_semaphores,
    # which splits multi-sem waits to satisfy the TRN2 1-wait-per-instruction
    # constraint that walrus codegen enforces.
    nc = bacc.Bacc(None, target_bir_lowering=False, debug=False)
    io = {
        "q": nc.declare_dram_parameter("q", [L, 16, 128], F32,
                                       isOutput=False).ap(),
        "kv": nc.declare_dram_parameter("kv", [L, 4, 257], F32,
                                        isOutput=False).ap(),
        "q_out": nc.declare_dram_parameter("q_out", [L, 16, 128], F32,
                                           isOutput=True).ap(),
        "k2": nc.declare_dram_parameter("k2", [L, 16, 128], F32,
                                        isOutput=True).ap(),
        "v2": nc.declare_dram_parameter("v2", [L, 16, 128], F32,
                                        isOutput=True).ap(),
        "mask": nc.declare_dram_parameter("mask", [L, 16], F32,
                                          isOutput=True).ap(),
    }
    with tile.TileContext(nc) as tc:
        _emit_core_kernel(tc, io, L=L)
    nc.finalize()
    _PROGRAM = nc
    return nc


def _shard_inputs(queries, keys, values):
    in_maps = []
    for c in range(N_CORES):
        b, half = c // 2, c % 2
        q_c = np.ascontiguousarray(
            queries[:, b, 16 * half:16 * (half + 1), :], dtype=np.float32)
        k_c = keys[:, b, 4 * half:4 * (half + 1), :]
        v_c = values[:, b, 4 * half:4 * (half + 1), :]
        kv_c = np.concatenate(
            [k_c, v_c, np.ones((L, 4, 1), np.float32)], axis=-1)
        in_maps.append({"q": q_c, "kv": np.ascontiguousarray(kv_c)})
    return in_maps


def _unshard_outputs(results):
    q_out = np.empty((L, B, Q_H, D), np.float32)
    k2 = np.empty((L, B, Q_H, D), np.float32)
    v2 = np.empty((L, B, Q_H, D), np.float32)
    mask = np.empty((L, B, Q_H), np.float32)
    for c in range(N_CORES):
        b, half = c // 2, c % 2
        sl = slice(16 * half, 16 * (half + 1))
        r = results[c]
        q_out[:, b, sl, :] = r["q_out"]
        k2[:, b, sl, :] = r["k2"]
        v2[:, b, sl, :] = r["v2"]
        mask[:, b, sl] = r["mask"]
    return q_out, k2, v2, mask


def run(queries, keys, values, **spmd_kwargs):
    """Run on 8 NeuronCores; returns ((q_out, k2, v2, mask), BassKernelResults)."""
    nc = _build_program()
    in_maps = _shard_inputs(queries, keys, values)
    br = run_bass_kernel_spmd(nc, in_maps, list(range(N_CORES)),
                              **spmd_kwargs)
    return _unshard_outputs(br.results), br


def kernel(queries, keys, values):
    outs, _ = run(queries, keys, values)
    return outs
